# revision 1
# baseline (speedup 1.0000x reference)
"""EnhancedLSTMCell Trainium2 kernel.

Data-parallel over 8 NeuronCores: batch B=8192 split into 8 shards of 1024
rows. Per core:
    gates = [x | h_prev] @ W + b          # [1024, 4096] via PE, fp32r
    i,f,g,o = split(gates); f *= mask
    c = f*c_prev + i*g; c = LayerNorm(c)*gamma + beta; h = o*tanh(c)

Layout: batch rows on partitions (8 chunks of 128), contraction dim K=2048 on
partitions for matmul operands. The host feeds [x | h_prev]^T per shard so
the contraction dim lands on partitions with unit-stride DMA; tiles are
rounded to fp32r (full-rate PE at free-dim >= 256) by DVE copies. W is
streamed once in 16 column-slices of 256, each as four pipelined quarter-K
chunks (fp32 staging -> fp32r via ACT) with a one-block prefetch queue.
Bias enters each PSUM accumulation via a leading K=1 ones-row matmul; ACT
consumes gate pre-activations straight from PSUM. c accumulates in SBUF: the i-drain
writes sigmoid(i) in place, the g-drain multiplies tanh(g) in, the f-drain
adds (sigmoid(f)*mask)*c_prev. LayerNorm uses bn_stats/bn_aggr + Sqrt +
DVE reciprocal; tanh(c_t) overwrites the accumulator to feed
h = sigmoid(o) * tanh(c_t).

Built on bacc.Bacc (not bass.Bass): Bacc's finalize() legalizes multi-sem
waits that the walrus DMA/LDW instruction encodings cannot carry.
"""

import sys

if "/opt/trn_rl_repo" not in sys.path:
    sys.path.insert(0, "/opt/trn_rl_repo")

import numpy as np

B = 8192
IN = 1024
H = 1024
NCORES = 8
BC = B // NCORES          # 1024 rows per core
MCH = BC // 128           # 8 partition chunks of batch rows
KCH = (IN + H) // 128     # 16 contraction chunks
CB = 256                  # W column-block width
EPS = 1e-5

_PROGRAMS = {}


def _build_program(trivial_gb: bool):
    from contextlib import ExitStack

    import concourse.bass as bass
    import concourse.tile as tile
    from concourse import bacc, mybir

    F32 = mybir.dt.float32
    F32R = mybir.dt.float32r
    AF = mybir.ActivationFunctionType
    ALU = mybir.AluOpType

    nc = bacc.Bacc("TRN2", target_bir_lowering=False, debug=False)

    # combined^T = [x | h_prev]^T per shard, transposed host-side during
    # sharding so the contraction dim lands on partitions with unit-stride DMA
    ct_d = nc.dram_tensor("combT", [IN + H, BC], F32, kind="ExternalInput").ap()
    c_d = nc.dram_tensor("c_prev", [BC, H], F32, kind="ExternalInput").ap()
    m_d = nc.dram_tensor("forget_mask", [MCH, 128], F32, kind="ExternalInput").ap()
    w_d = nc.dram_tensor("W", [IN + H, 4 * H], F32, kind="ExternalInput").ap()
    b_d = nc.dram_tensor("b", [1, 4 * H], F32, kind="ExternalInput").ap()
    g_d = nc.dram_tensor("ln_gamma", [1, H], F32, kind="ExternalInput").ap()
    be_d = nc.dram_tensor("ln_beta", [1, H], F32, kind="ExternalInput").ap()
    ho_d = nc.dram_tensor("h_out", [BC, H], F32, kind="ExternalOutput").ap()
    co_d = nc.dram_tensor("c_out", [BC, H], F32, kind="ExternalOutput").ap()

    w_k = w_d.rearrange("(k p) n -> p k n", p=128)  # [128, 16, 4096]
    ct_k = ct_d.rearrange("(k p) b -> p k b", p=128)  # [128, 16, 1024]

    with tile.TileContext(nc) as tc, ExitStack() as ctx:
        singles = ctx.enter_context(tc.tile_pool(name="singles", bufs=1))
        bigs = ctx.enter_context(tc.tile_pool(name="bigs", bufs=1))
        wpool = ctx.enter_context(tc.tile_pool(name="w", bufs=4))
        wrpool = ctx.enter_context(tc.tile_pool(name="wr", bufs=2))
        ctpool = ctx.enter_context(tc.tile_pool(name="ctstage", bufs=2))
        tpool = ctx.enter_context(tc.tile_pool(name="tmp", bufs=4))
        cppool = ctx.enter_context(tc.tile_pool(name="cprev", bufs=3))
        hpool = ctx.enter_context(tc.tile_pool(name="hout", bufs=3))
        zpool = ctx.enter_context(
            tc.tile_pool(name="z", bufs=2 if trivial_gb else 1))
        pmain = ctx.enter_context(tc.tile_pool(name="pmain", bufs=8, space="PSUM"))

        # bias enters PSUM via a K=1 ones-row matmul (start=True), so the
        # k-loop accumulates on top and ACT drains see gates+bias directly.
        # (A DVE post-add was measured slower: it sits in every psum->drain
        # chain, costing more than the 14us of PE the ones-matmuls use.)
        ones_r = singles.tile([1, 128], F32R)
        b_r = singles.tile([1, 4 * H], F32R)
        with tc.tile_pool(name="stage", bufs=1) as stage:
            ones_f = stage.tile([1, 128], F32)
            nc.vector.memset(ones_f, 1.0)
            nc.scalar.copy(ones_r, ones_f)
            b_stage = stage.tile([1, 4 * H], F32)
            nc.sync.dma_start(out=b_stage, in_=b_d)
            nc.scalar.copy(b_r, b_stage)
        mask_sb = singles.tile([128, MCH], F32)
        nc.sync.dma_start(out=mask_sb, in_=m_d.rearrange("m p -> p m"))
        if not trivial_gb:
            gam_bc = singles.tile([128, H], F32)
            nc.sync.dma_start(
                out=gam_bc,
                in_=bass.AP(tensor=g_d.tensor, offset=g_d.offset,
                            ap=[[0, 128], g_d.ap[1]]),
            )
            bet_bc = singles.tile([128, H], F32)
            nc.sync.dma_start(
                out=bet_bc,
                in_=bass.AP(tensor=be_d.tensor, offset=be_d.offset,
                            ap=[[0, 128], be_d.ap[1]]),
            )

        # combT[k, m] = (128x128 transposed block of [x | h_prev]), stored
        # pre-rounded to fp32r for the PE
        combT = bigs.tile([128, KCH, MCH, 128], F32R)
        c_acc = bigs.tile([128, MCH, H], F32)
        mvall = singles.tile([128, MCH, 2], F32)
        std_t = singles.tile([128, MCH], F32)
        inv_t = singles.tile([128, MCH], F32)
        eps_t = singles.tile([128, 1], F32)
        nc.vector.memset(eps_t, EPS)

        # ---- main loop over W column blocks ----
        # order: (i,g) interleaved per quarter, then f, then LN, then o
        GOFF = {"i": 0, "f": H, "g": 2 * H, "o": 3 * H}
        NQ = H // CB  # quarters per gate
        blocks = []
        for q in range(NQ):
            blocks.append(("i", q))
            blocks.append(("g", q))
        blocks += [("f", q) for q in range(NQ)]
        o_blocks = [("o", q) for q in range(NQ)]

        def load_w(gate, q):
            # W slice load in four pipelined quarter-K chunks: matmuls for
            # the first k-tiles start as soon as the first chunk is rounded,
            # and chunk DMAs of the next block overlap the current block.
            col0 = GOFF[gate] + q * CB
            wr = wrpool.tile([128, KCH, CB], F32R, tag="wr")
            hk = KCH // 4
            for hchunk in range(4):
                wt = wpool.tile([128, hk, CB], F32, tag="w")
                nc.sync.dma_start(
                    out=wt,
                    in_=w_k[:, hchunk * hk:(hchunk + 1) * hk, col0:col0 + CB])
                # fp32 -> fp32r rounding on ACT
                nc.scalar.copy(wr[:, hchunk * hk:(hchunk + 1) * hk, :], wt)
            return wr

        def do_block(gate, q, wr):
            col0 = GOFF[gate] + q * CB
            for m in range(MCH):
                ps = pmain.tile([128, CB], F32, tag="ps")
                nc.tensor.matmul(ps, ones_r, b_r[:, col0:col0 + CB],
                                 start=True, stop=False)
                for k in range(KCH):
                    nc.tensor.matmul(
                        ps, combT[:, k, m, :], wr[:, k, :],
                        start=False, stop=(k == KCH - 1),
                    )
                csl = c_acc[:, m, q * CB:(q + 1) * CB]
                if gate == "i":
                    nc.scalar.activation(csl, ps, AF.Sigmoid)
                elif gate == "g":
                    tg = tpool.tile([128, CB], F32, tag="t")
                    nc.scalar.activation(tg, ps, AF.Tanh)
                    nc.vector.tensor_mul(csl, csl, tg)
                elif gate == "f":
                    tf = tpool.tile([128, CB], F32, tag="t")
                    nc.scalar.activation(tf, ps, AF.Sigmoid)
                    cp = cppool.tile([128, CB], F32, tag="cp")
                    nc.sync.dma_start(
                        out=cp,
                        in_=c_d[m * 128:(m + 1) * 128, q * CB:(q + 1) * CB])
                    t2 = tpool.tile([128, CB], F32, tag="t")
                    nc.vector.scalar_tensor_tensor(
                        t2, tf, mask_sb[:, m:m + 1], cp, ALU.mult, ALU.mult)
                    nc.vector.tensor_add(csl, csl, t2)
                else:  # o
                    to = tpool.tile([128, CB], F32, tag="t")
                    nc.scalar.activation(to, ps, AF.Sigmoid)
                    hh = hpool.tile([128, CB], F32, tag="h")
                    nc.vector.tensor_mul(hh, to, csl)  # csl holds tanh(c_t)
                    nc.sync.dma_start(
                        out=ho_d[m * 128:(m + 1) * 128, q * CB:(q + 1) * CB],
                        in_=hh)

        def emit_ln():
            # ---- LayerNorm over H per m-chunk ----
            for m in range(MCH):
                st = tpool.tile([128, 2, 6], F32, tag="st")
                for hf in range(2):
                    nc.vector.bn_stats(
                        out=st[:, hf, :],
                        in_=c_acc[:, m, hf * 512:(hf + 1) * 512])
                nc.vector.bn_aggr(out=mvall[:, m, :], in_=st)
            # std = sqrt(var + eps); inv = 1/std; nmi = -mean*inv
            nc.scalar.activation(std_t, mvall[:, :, 1], AF.Sqrt, bias=eps_t)
            nc.vector.reciprocal(inv_t, std_t)
            for m in range(MCH):
                z = zpool.tile([128, H], F32, tag="z")
                nc.vector.tensor_scalar(
                    z, c_acc[:, m, :], mvall[:, m, 0:1], inv_t[:, m:m + 1],
                    ALU.subtract, ALU.mult)
                if not trivial_gb:
                    nc.vector.tensor_mul(z, z, gam_bc)
                    nc.vector.tensor_add(z, z, bet_bc)
                nc.sync.dma_start(out=co_d[m * 128:(m + 1) * 128, :], in_=z)
                nc.scalar.activation(c_acc[:, m, :], z, AF.Tanh)

        # One-block W prefetch: the next block's W chunks (DMA + fp32r
        # rounding) are emitted before the current block's matmuls, and the
        # first o-block's W is already in flight before the LayerNorm work.
        allb = blocks + o_blocks

        # ---- load combined^T and round to fp32r; the m=0 slab goes first,
        # then the first W slice, then the remaining slabs, so the serial
        # DMA startup chain covers exactly what the first matmuls need ----
        hk0 = KCH // 2

        def load_ct(m):
            for hchunk in range(2):
                cts = ctpool.tile([128, hk0, 128], F32, tag="cts")
                nc.sync.dma_start(
                    out=cts,
                    in_=ct_k[:, hchunk * hk0:(hchunk + 1) * hk0,
                             m * 128:(m + 1) * 128])
                nc.vector.tensor_copy(
                    combT[:, hchunk * hk0:(hchunk + 1) * hk0, m, :], cts)

        load_ct(0)
        wr_next = load_w(*allb[0])
        for m in range(1, MCH):
            load_ct(m)

        ln_done = False
        for idx, (gate, q) in enumerate(allb):
            if gate == "o" and not ln_done:
                emit_ln()
                ln_done = True
            wr_cur = wr_next
            if idx + 1 < len(allb):
                wr_next = load_w(*allb[idx + 1])
            do_block(gate, q, wr_cur)

    nc.finalize()
    return nc


def _get_program(trivial_gb: bool):
    if trivial_gb not in _PROGRAMS:
        _PROGRAMS[trivial_gb] = _build_program(trivial_gb)
    return _PROGRAMS[trivial_gb]


def kernel(x, h_prev, c_prev, forget_mask, W, b, ln_gamma, ln_beta):
    from concourse.bass_utils import run_bass_kernel_spmd

    f32 = np.float32
    x = np.ascontiguousarray(x, dtype=f32)
    h_prev = np.ascontiguousarray(h_prev, dtype=f32)
    c_prev = np.ascontiguousarray(c_prev, dtype=f32)
    forget_mask = np.ascontiguousarray(forget_mask, dtype=f32)
    W = np.ascontiguousarray(W, dtype=f32)
    b = np.ascontiguousarray(b, dtype=f32)
    ln_gamma = np.ascontiguousarray(ln_gamma, dtype=f32)
    ln_beta = np.ascontiguousarray(ln_beta, dtype=f32)

    trivial_gb = bool(np.all(ln_gamma == 1.0) and np.all(ln_beta == 0.0))
    nc = _get_program(trivial_gb)

    # pre-transposed [x | h_prev] per shard: [IN+H, BC], contraction-major
    comb_t = np.ascontiguousarray(
        np.concatenate((x, h_prev), axis=1).T)  # [IN+H, B]

    in_maps = []
    for i in range(NCORES):
        sl = slice(i * BC, (i + 1) * BC)
        in_maps.append({
            "combT": np.ascontiguousarray(comb_t[:, sl]),
            "c_prev": c_prev[sl],
            "forget_mask": forget_mask[sl].reshape(MCH, 128),
            "W": W,
            "b": b.reshape(1, 4 * H),
            "ln_gamma": ln_gamma.reshape(1, H),
            "ln_beta": ln_beta.reshape(1, H),
        })

    res = run_bass_kernel_spmd(nc, in_maps, list(range(NCORES)))
    h_t = np.concatenate([r["h_out"] for r in res.results], axis=0)
    c_t = np.concatenate([r["c_out"] for r in res.results], axis=0)
    return (h_t, c_t)



# revision 6
# speedup vs baseline: 1.3075x; 1.3075x over previous
"""EnhancedLSTMCell Trainium2 kernel — fp8 DoubleRow 3-pass GEMM.

Data-parallel over 8 NeuronCores: batch B=8192 split into 8 shards of 1024
rows. Per core:
    gates = [x | h_prev] @ W + b          # [1024, 4096]
    i,f,g,o = split(gates); f *= mask
    c = f*c_prev + i*g; c = LayerNorm(c)*gamma + beta; h = o*tanh(c)

The GEMM runs on the PE in fp8e4 (e4m3) DoubleRow perf mode: each matmul
instruction contracts TWO 128-deep K-tiles ([128, 2, M] stationary x
[128, 2, N] moving) at 0.5 cycles per output row — 4x the fp32r rate.
Straight e4m3 is too coarse for the 2e-2 gate (max-relerr ~0.19), so the
product is computed in three telescoping passes that cancel first-order
quantization noise (measured relerr_h ~ 6e-3):

    a8  = e4m3(a)            W8  = e4m3(256 W)
    ar8 = e4m3(16 (a-a8))    W8b = e4m3(16 W)      Wr8 = e4m3(256 W - W8)
    PSUM = a8@W8 + ar8@W8b + a8@Wr8 + 256 b  ==  256 (a W + b) + O(eps^2)

All three passes accumulate into one PSUM group per (m-chunk, 512-col
block); the x256 scale is divided out for free by the ACT drain's
`scale=` operand. Bias enters via a K=1 ones-pair DoubleRow matmul.

Quantization, transposition and tiling all happen host-side: operands are
DMA'd as pre-packed fp8 images whose per-partition runs are >=512B
contiguous (full modeled DMA bandwidth), c_prev and both outputs travel as
bf16. Per core that is ~34MB of HBM traffic vs ~170us of PE work, so the
kernel stays PE-bound at the fp8 roofline.

Drain pipeline (per 512-col block, 8 m-chunks): ACT applies
sigmoid/tanh straight from PSUM; DVE folds c = sig(f)*mask*c_prev +
sig(i)*tanh(g); LayerNorm uses bn_stats/bn_aggr + Sqrt + reciprocal;
h = sig(o)*tanh(c_t) with bf16 tiles DMA'd straight out.

Built on bacc.Bacc (not bass.Bass): Bacc's finalize() legalizes multi-sem
waits that the walrus DMA/LDW instruction encodings cannot carry.
"""

import sys

if "/opt/trn_rl_repo" not in sys.path:
    sys.path.insert(0, "/opt/trn_rl_repo")

import ml_dtypes
import numpy as np

B = 8192
IN = 1024
H = 1024
NCORES = 8
BC = B // NCORES          # 1024 rows per core
MCH = BC // 128           # 8 partition chunks of batch rows
K = IN + H                # 2048 contraction
KCH = K // 128            # 16 K-tiles
KP = KCH // 2             # 8 DoubleRow K-pairs
CB = 512                  # W column-block width (one PSUM bank)
NB = 4 * H // CB          # 8 column blocks
EPS = 1e-5
SW = 256.0                # base W / bias / PSUM scale
SAR = 16.0                # a-residual quantization scale
SWB = 16.0                # pass-2 W image scale (SAR * SWB == SW)
SINV = 1.0 / SW

E4NP = ml_dtypes.float8_e4m3
BFNP = ml_dtypes.bfloat16

_PROGRAMS = {}


def _build_program(trivial_gb: bool):
    from contextlib import ExitStack

    import concourse.bass as bass
    import concourse.tile as tile
    from concourse import bacc, mybir

    F32 = mybir.dt.float32
    F8 = mybir.dt.float8e4
    BF = mybir.dt.bfloat16
    AF = mybir.ActivationFunctionType
    ALU = mybir.AluOpType
    DR = mybir.MatmulPerfMode.DoubleRow

    nc = bacc.Bacc("TRN2", target_bir_lowering=False, debug=False)

    a8_d = nc.dram_tensor("a8T", [128, MCH, KCH, 128], F8, kind="ExternalInput").ap()
    ar8_d = nc.dram_tensor("ar8T", [128, MCH, KCH, 128], F8, kind="ExternalInput").ap()
    w8_d = nc.dram_tensor("w8", [128, NB, KCH, CB], F8, kind="ExternalInput").ap()
    w8b_d = nc.dram_tensor("w8b", [128, NB, KCH, CB], F8, kind="ExternalInput").ap()
    wr8_d = nc.dram_tensor("wr8", [128, NB, KCH, CB], F8, kind="ExternalInput").ap()
    bz_d = nc.dram_tensor("bz", [1, 2, 4 * H], F8, kind="ExternalInput").ap()
    m_d = nc.dram_tensor("forget_mask", [MCH, 128], F32, kind="ExternalInput").ap()
    cp_d = nc.dram_tensor("c_prev", [128, MCH, H], BF, kind="ExternalInput").ap()
    g_d = nc.dram_tensor("ln_gamma", [1, H], F32, kind="ExternalInput").ap()
    be_d = nc.dram_tensor("ln_beta", [1, H], F32, kind="ExternalInput").ap()
    ho_d = nc.dram_tensor("h_out", [BC, H], BF, kind="ExternalOutput").ap()
    co_d = nc.dram_tensor("c_out", [BC, H], BF, kind="ExternalOutput").ap()

    with tile.TileContext(nc) as tc, ExitStack() as ctx:
        singles = ctx.enter_context(tc.tile_pool(name="singles", bufs=1))
        bigs = ctx.enter_context(tc.tile_pool(name="bigs", bufs=1))
        wpool = ctx.enter_context(tc.tile_pool(name="w", bufs=2))
        tpool = ctx.enter_context(tc.tile_pool(name="tmp", bufs=6))
        hpool = ctx.enter_context(tc.tile_pool(name="hout", bufs=3))
        zpool = ctx.enter_context(tc.tile_pool(name="z", bufs=2))
        pmain = ctx.enter_context(tc.tile_pool(name="pmain", bufs=8, space="PSUM"))

        ones8 = singles.tile([1, 2, 128], F8)
        nc.vector.memset(ones8, 1.0)
        bz_sb = singles.tile([1, 2, 4 * H], F8)
        nc.sync.dma_start(out=bz_sb, in_=bz_d)
        mask_sb = singles.tile([128, MCH], F32)
        nc.sync.dma_start(out=mask_sb, in_=m_d.rearrange("m p -> p m"))
        eps_t = singles.tile([128, 1], F32)
        nc.vector.memset(eps_t, EPS)
        if not trivial_gb:
            gam_bc = singles.tile([128, H], F32)
            nc.sync.dma_start(
                out=gam_bc,
                in_=bass.AP(tensor=g_d.tensor, offset=g_d.offset,
                            ap=[[0, 128], g_d.ap[1]]),
            )
            bet_bc = singles.tile([128, H], F32)
            nc.sync.dma_start(
                out=bet_bc,
                in_=bass.AP(tensor=be_d.tensor, offset=be_d.offset,
                            ap=[[0, 128], be_d.ap[1]]),
            )

        c_acc = bigs.tile([128, MCH, H], F32)
        cp_sb = bigs.tile([128, MCH, H], BF)
        mvall = singles.tile([128, MCH, 2], F32)
        std_t = singles.tile([128, MCH], F32)
        inv_t = singles.tile([128, MCH], F32)

        a8_sb = [bigs.tile([128, KCH, 128], F8, tag=f"a8_{m}", name=f"a8_{m}")
                 for m in range(MCH)]
        ar8_sb = [bigs.tile([128, KCH, 128], F8, tag=f"ar8_{m}", name=f"ar8_{m}")
                  for m in range(MCH)]

        def load_a(m):
            nc.sync.dma_start(out=a8_sb[m], in_=a8_d[:, m:m + 1, :, :])

        def load_ar(m):
            nc.sync.dma_start(out=ar8_sb[m], in_=ar8_d[:, m:m + 1, :, :])

        NBI = {"i": 0, "f": 2, "g": 4, "o": 6}
        W_IMGS = (("w8", w8_d), ("w8b", w8b_d), ("wr8", wr8_d))

        def load_w_img(gate, q, img_idx):
            nb = NBI[gate] + q
            tag, img = W_IMGS[img_idx]
            wt = wpool.tile([128, KCH, CB], F8, tag=tag)
            nc.sync.dma_start(out=wt, in_=img[:, nb:nb + 1, :, :])
            return wt

        def load_w(gate, q):
            return [load_w_img(gate, q, i) for i in range(3)]

        def emit_pass(ps, m, pi, wt, stop):
            at = ar8_sb[m] if pi == 1 else a8_sb[m]
            for kp in range(KP):
                nc.tensor.matmul(
                    ps, at[:, 2 * kp:2 * kp + 2, :],
                    wt[:, 2 * kp:2 * kp + 2, :],
                    start=False, stop=(stop and kp == KP - 1),
                    perf_mode=DR)

        def do_block(gate, q, wts, pass_major=False):
            w8t, w8bt, wr8t = wts
            nb = NBI[gate] + q
            col0 = nb * CB
            pss = []
            if pass_major:
                # first block: run pass p over all m before pass p+1 so the
                # pass-1 matmuls overlap the in-flight w8b/wr8 (+ar8) DMAs
                for m in range(MCH):
                    ps = pmain.tile([128, CB], F32, tag="ps")
                    nc.tensor.matmul(ps, ones8, bz_sb[:, :, col0:col0 + CB],
                                     start=True, stop=False, perf_mode=DR)
                    emit_pass(ps, m, 0, w8t, False)
                    pss.append(ps)
                for m in range(MCH):
                    emit_pass(pss[m], m, 1, w8bt, False)
                for m in range(MCH):
                    emit_pass(pss[m], m, 2, wr8t, True)
            for m in range(MCH):
                if pass_major:
                    ps = pss[m]
                else:
                    ps = pmain.tile([128, CB], F32, tag="ps")
                    # bias: ones-pair x [b8; 0] outer product, DoubleRow
                    nc.tensor.matmul(ps, ones8, bz_sb[:, :, col0:col0 + CB],
                                     start=True, stop=False, perf_mode=DR)
                    emit_pass(ps, m, 0, w8t, False)
                    emit_pass(ps, m, 1, w8bt, False)
                    emit_pass(ps, m, 2, wr8t, True)
                csl = c_acc[:, m, q * CB:(q + 1) * CB]
                if gate == "i":
                    nc.scalar.activation(csl, ps, AF.Sigmoid, scale=SINV)
                elif gate == "g":
                    tg = tpool.tile([128, CB], F32, tag="t")
                    nc.scalar.activation(tg, ps, AF.Tanh, scale=SINV)
                    nc.vector.tensor_mul(csl, csl, tg)
                elif gate == "f":
                    tf = tpool.tile([128, CB], F32, tag="t")
                    nc.scalar.activation(tf, ps, AF.Sigmoid, scale=SINV)
                    t2 = tpool.tile([128, CB], F32, tag="t")
                    nc.vector.scalar_tensor_tensor(
                        t2, tf, mask_sb[:, m:m + 1],
                        cp_sb[:, m, q * CB:(q + 1) * CB], ALU.mult, ALU.mult)
                    nc.vector.tensor_add(csl, csl, t2)
                else:  # o
                    to = tpool.tile([128, CB], F32, tag="t")
                    nc.scalar.activation(to, ps, AF.Sigmoid, scale=SINV)
                    hh = hpool.tile([128, CB], BF, tag="h")
                    nc.vector.tensor_mul(hh, to, csl)  # csl holds tanh(c_t)
                    nc.sync.dma_start(
                        out=ho_d[m * 128:(m + 1) * 128, q * CB:(q + 1) * CB],
                        in_=hh)

        def emit_ln():
            for m in range(MCH):
                st = tpool.tile([128, 2, 6], F32, tag="st")
                for hf in range(2):
                    nc.vector.bn_stats(
                        out=st[:, hf, :],
                        in_=c_acc[:, m, hf * 512:(hf + 1) * 512])
                nc.vector.bn_aggr(out=mvall[:, m, :], in_=st)
            nc.scalar.activation(std_t, mvall[:, :, 1], AF.Sqrt, bias=eps_t)
            nc.vector.reciprocal(inv_t, std_t)
            for m in range(MCH):
                z = zpool.tile([128, H], BF, tag="z")
                nc.vector.tensor_scalar(
                    z, c_acc[:, m, :], mvall[:, m, 0:1], inv_t[:, m:m + 1],
                    ALU.subtract, ALU.mult)
                if not trivial_gb:
                    nc.vector.tensor_mul(z, z, gam_bc)
                    nc.vector.tensor_add(z, z, bet_bc)
                nc.sync.dma_start(out=co_d[m * 128:(m + 1) * 128, :], in_=z)
                nc.scalar.activation(c_acc[:, m, :], z, AF.Tanh)

        blocks = [("i", 0), ("g", 0), ("i", 1), ("g", 1), ("f", 0), ("f", 1)]
        o_blocks = [("o", 0), ("o", 1)]
        allb = blocks + o_blocks

        # startup DMA order paces arrivals against the pass-major first
        # block: a8_0 + w8(b0) gate pass-1, which then covers the w8b, ar8
        # and wr8 streams
        load_a(0)
        w8t0 = load_w_img(*allb[0], 0)
        for m in range(1, MCH):
            load_a(m)
        w8bt0 = load_w_img(*allb[0], 1)
        for m in range(MCH):
            load_ar(m)
        wr8t0 = load_w_img(*allb[0], 2)
        wts_next = [w8t0, w8bt0, wr8t0]

        ln_done = False
        for idx, (gate, q) in enumerate(allb):
            wts_cur = wts_next
            if idx + 1 < len(allb):
                wts_next = load_w(*allb[idx + 1])
            if idx == 1:
                for m in range(0, 4):
                    nc.sync.dma_start(out=cp_sb[:, m:m + 1, :],
                                      in_=cp_d[:, m:m + 1, :])
            elif idx == 2:
                for m in range(4, MCH):
                    nc.sync.dma_start(out=cp_sb[:, m:m + 1, :],
                                      in_=cp_d[:, m:m + 1, :])
            if gate == "o" and not ln_done:
                emit_ln()
                ln_done = True
            do_block(gate, q, wts_cur, pass_major=(idx == 0))

    nc.finalize()
    return nc


def _get_program(trivial_gb: bool):
    if trivial_gb not in _PROGRAMS:
        _PROGRAMS[trivial_gb] = _build_program(trivial_gb)
    return _PROGRAMS[trivial_gb]


def prep_inputs(x, h_prev, c_prev, forget_mask, W, b, ln_gamma, ln_beta):
    """Host-side quantization + tiling. Returns (in_maps, trivial_gb)."""
    f32 = np.float32
    x = np.ascontiguousarray(x, dtype=f32)
    h_prev = np.ascontiguousarray(h_prev, dtype=f32)
    c_prev = np.ascontiguousarray(c_prev, dtype=f32)
    forget_mask = np.ascontiguousarray(forget_mask, dtype=f32)
    W = np.ascontiguousarray(W, dtype=f32)
    b = np.ascontiguousarray(b, dtype=f32)
    ln_gamma = np.ascontiguousarray(ln_gamma, dtype=f32)
    ln_beta = np.ascontiguousarray(ln_beta, dtype=f32)

    trivial_gb = bool(np.all(ln_gamma == 1.0) and np.all(ln_beta == 0.0))

    a = np.concatenate((x, h_prev), axis=1)          # [B, K]
    a8 = a.astype(E4NP)
    ar8 = ((a - a8.astype(f32)) * SAR).astype(E4NP)
    W8 = (W * SW).astype(E4NP)
    Wr8 = (W * SW - W8.astype(f32)).astype(E4NP)
    W8b = (W * SWB).astype(E4NP)

    def pack_w(Wq):  # [K, 4H] -> [128, NB, KCH, CB]
        return np.ascontiguousarray(
            Wq.reshape(KCH, 128, NB, CB).transpose(1, 2, 0, 3))

    w8p, w8bp, wr8p = pack_w(W8), pack_w(W8b), pack_w(Wr8)

    bz = np.zeros((1, 2, 4 * H), dtype=E4NP)
    bz[0, 0, :] = (b * SW).astype(E4NP)

    def pack_a(Aq, i):  # [B, K] -> per-core [128, MCH, KCH, 128]
        As = Aq[i * BC:(i + 1) * BC].reshape(MCH, 128, KCH, 128)
        return np.ascontiguousarray(As.transpose(3, 0, 2, 1))

    cpb = c_prev.astype(BFNP)
    gam = ln_gamma.reshape(1, H)
    bet = ln_beta.reshape(1, H)

    in_maps = []
    for i in range(NCORES):
        sl = slice(i * BC, (i + 1) * BC)
        cp_i = np.ascontiguousarray(
            cpb[sl].reshape(MCH, 128, H).transpose(1, 0, 2))
        in_maps.append({
            "a8T": pack_a(a8, i),
            "ar8T": pack_a(ar8, i),
            "w8": w8p, "w8b": w8bp, "wr8": wr8p,
            "bz": bz,
            "forget_mask": forget_mask[sl].reshape(MCH, 128),
            "c_prev": cp_i,
            "ln_gamma": gam,
            "ln_beta": bet,
        })
    return in_maps, trivial_gb


def kernel(x, h_prev, c_prev, forget_mask, W, b, ln_gamma, ln_beta):
    from concourse.bass_utils import run_bass_kernel_spmd

    in_maps, trivial_gb = prep_inputs(
        x, h_prev, c_prev, forget_mask, W, b, ln_gamma, ln_beta)
    nc = _get_program(trivial_gb)

    res = run_bass_kernel_spmd(nc, in_maps, list(range(NCORES)))
    f32 = np.float32
    h_t = np.concatenate(
        [np.asarray(r["h_out"]).astype(f32) for r in res.results], axis=0)
    c_t = np.concatenate(
        [np.asarray(r["c_out"]).astype(f32) for r in res.results], axis=0)
    return (h_t, c_t)
